# revision 40
# baseline (speedup 1.0000x reference)
"""Kuramoto oscillator network kernel for 8 Trainium2 NeuronCores.

Problem: B=256 batches, D=256 feature dims, N=16 oscillator dims, T=25 steps.
    c = emb[:,:,None]*W_d + b_d                        [B,D,N]
    x = normalize(noise + c)                            (init, per (b,d) over N)
    repeat T: f1 = J_in@x1 + J_out@x2 + c1  (einsum ijkl,bjl->bik)
              p  = f - <x,f>x ; om = Omega@x
              x  = normalize(x + g*(om + p))
    out = stack(x1, x2)                                 [2,B,D,N]

Strategy (v2): sum/difference symmetrization + transposed matmul geometry.
  * With s=x1+x2, d=x1-x2, A=(J_in+J_out)/2, Bm=(J_in-J_out)/2:
        f1 = A s + Bm d,  f2 = A s - Bm d
    -- HALF the matmul FLOPs of the naive 4-einsum form.
  * Omega rotation is skew-symmetric (<x,Omega x>=0) so it can be folded
    into A and Bm on the host (block-diagonal add) -- zero device cost.
  * The conditional stimulus c enters f every step; (c1+c2)/2 and
    (c1-c2)/2 are rank-structured (emb (x) W_d) and are folded into the
    same PSUM accumulation as 2 extra matmul chunks (emb^T stationary,
    block-diag W moving).
  * Transposed geometry: stationary = gathered state chunk [jl=128, b=128],
    moving = A/Bm column slice [128, 512] (fp16, N=512 per matmul).
    Output lands batch-major [b, ik], which makes the per-(b,i) reductions
    (projection <x,f> and the normalize norm) native windowed DVE reduces
    (shape [128, 32, 16] axis-X), eliminating the block-ones matmuls and
    the giant broadcast reciprocal of v1.
  * Model-parallel over ik (each core owns 512 of 4096 ik), batch kept
    whole per matmul; batches split in 2 groups only for AllGather/compute
    pipelining. Per step per group: 64 matmuls N=512 + 3 c-fold matmuls +
    8 PE transposes (to return x' to [jl, b] layout for the AllGather).
  * Elementwise work is split across Vector (reduces + state-1 chain),
    GpSimd (state-2 chain), Scalar/ACT (squares, sqrt, affine).

Self-contained: hardcodes shapes; no imports from /root/problem.
"""

import os
import sys
import time

sys.path.insert(0, "/opt/trn_rl_repo")

import numpy as np

import concourse.bass as bass
import concourse.mybir as mybir
import concourse.tile as tile
from concourse import bacc
from concourse import bass2jax
from concourse.bass_interp import get_hw_module

B, D, N = 256, 256, 16
DN = D * N                      # 4096 flattened (i,k) / (j,l)
T = int(os.environ.get("KUR_T", "25"))
GAMMA = 0.1
NCORES = 8
IK = DN // NCORES               # 512 ik per core (32 i values)
NI = D // NCORES                # 32 i per core
BT = 128                        # batches per group
NG = 2                          # batch groups
NCH = DN // 128                 # 32 contraction chunks

FP32 = mybir.dt.float32
FP16 = mybir.dt.float16

_CACHE = {}


def _build(nc):
    AF = mybir.ActivationFunctionType
    ALU = mybir.AluOpType

    # ---------------- DRAM I/O ----------------
    a_d = nc.dram_tensor("a_mat", [DN, IK], FP16, kind="ExternalInput")
    b_d = nc.dram_tensor("b_mat", [DN, IK], FP16, kind="ExternalInput")
    wbd_d = nc.dram_tensor("wbd", [128, IK], FP16, kind="ExternalInput")
    # per (group, kind): kind = [emb_s, emb_d, emb1, emb2] chunks, each [128,128]
    embs_d = nc.dram_tensor("embs", [128, NG * 4 * 128], FP16, kind="ExternalInput")
    ones_d = nc.dram_tensor("ones_r", [1, 128], FP16, kind="ExternalInput")
    bdr_d = nc.dram_tensor("bd_r", [1, IK], FP16, kind="ExternalInput")
    id_d = nc.dram_tensor("ident", [128, 128], FP16, kind="ExternalInput")
    noise_d = nc.dram_tensor("noise", [B, 2 * IK], FP32, kind="ExternalInput")
    out_d = nc.dram_tensor("xt_out", [B, 2 * IK], FP32, kind="ExternalOutput")

    # internal HBM: AG input (own transposed slice) + gathered state, x2 parity
    agin = [
        [nc.dram_tensor(f"agin{g}_{p}", [IK, 256], FP16) for p in range(2)]
        for g in range(NG)
    ]
    xg = [
        [
            nc.dram_tensor(f"xg{g}_{p}", [DN, 256], FP16, addr_space="Shared")
            for p in range(2)
        ]
        for g in range(NG)
    ]

    V = nc.vector
    P = nc.gpsimd
    S = nc.scalar

    with tile.TileContext(nc) as tc:
        with (
            tc.tile_pool(name="res", bufs=1) as res,
            tc.tile_pool(name="xgq", bufs=2) as xgq,
            tc.tile_pool(name="tmp", bufs=2) as tmp,
            tc.tile_pool(name="sd", bufs=2) as sdp,
            tc.tile_pool(name="fps", bufs=1, space="PSUM") as fps,
            tc.tile_pool(name="tps", bufs=1, space="PSUM") as tps,
        ):
            # ---------------- resident SBUF ----------------
            a_sb = res.tile([128, NCH * IK], FP16, tag="a")
            b_sb = res.tile([128, NCH * IK], FP16, tag="b")
            for k in range(NCH):
                nc.sync.dma_start(
                    out=a_sb[:, k * IK:(k + 1) * IK],
                    in_=a_d[k * 128:(k + 1) * 128, :],
                )
                nc.sync.dma_start(
                    out=b_sb[:, k * IK:(k + 1) * IK],
                    in_=b_d[k * 128:(k + 1) * 128, :],
                )
            wbd_sb = res.tile([128, IK], FP16, tag="wbd")
            nc.sync.dma_start(out=wbd_sb[:, :], in_=wbd_d[:, :])
            embs_sb = res.tile([128, NG * 4 * 128], FP16, tag="embs")
            nc.sync.dma_start(out=embs_sb[:, :], in_=embs_d[:, :])
            ones_sb = res.tile([1, 128], FP16, tag="ones")
            nc.sync.dma_start(out=ones_sb[:, :], in_=ones_d[:, :])
            bdr_sb = res.tile([1, IK], FP16, tag="bdr")
            nc.sync.dma_start(out=bdr_sb[:, :], in_=bdr_d[:, :])
            id_sb = res.tile([128, 128], FP16, tag="ident")
            nc.sync.dma_start(out=id_sb[:, :], in_=id_d[:, :])

            # local state x[g][st]: [128 b, 512 ik] fp32
            xs = [
                [
                    res.tile(
                        [128, IK], FP32, tag=f"x{g}_{st}", name=f"x{g}_{st}"
                    )
                    for st in range(2)
                ]
                for g in range(NG)
            ]

            prev_cc = [[None, None] for _ in range(NG)]
            prev_din = [[[], []] for _ in range(NG)]
            cur_q = [[None] * 4 for _ in range(NG)]

            def emb_chunk(g, kind):
                c0 = (g * 4 + kind) * 128
                return embs_sb[:, c0:c0 + 128]

            def win3(ap2d):
                """[128, 512] AP -> [128, 32, 16] windowed view."""
                return ap2d.rearrange("p (i k) -> p i k", k=16)

            def bcast(ap_small):
                """[128, 32] AP -> [128, 32, 16] broadcast view."""
                return ap_small[:, :, None].broadcast_to([128, NI, 16])

            def launch_gather(g, t):
                """s/d from x tiles -> PE transpose -> agin -> AllGather -> SBUF."""
                p = t % 2
                s_t = sdp.tile([128, IK], FP16, tag=f"s{g}")
                d_t = sdp.tile([128, IK], FP16, tag=f"d{g}")
                V.tensor_add(out=s_t[:, :], in0=xs[g][0][:, :], in1=xs[g][1][:, :])
                P.tensor_sub(out=d_t[:, :], in0=xs[g][0][:, :], in1=xs[g][1][:, :])
                tp = tps.tile([128, 8 * 128], FP16, tag=f"tp{g}")
                for m in range(4):
                    nc.tensor.transpose(
                        tp[:, (2 * m) * 128:(2 * m + 1) * 128],
                        s_t[:, m * 128:(m + 1) * 128],
                        id_sb[:, :],
                    )
                    nc.tensor.transpose(
                        tp[:, (2 * m + 1) * 128:(2 * m + 2) * 128],
                        d_t[:, m * 128:(m + 1) * 128],
                        id_sb[:, :],
                    )
                agst = sdp.tile([128, 8 * 128], FP16, tag=f"ag{g}", name=f"ag{g}")
                S.copy(agst[:, :], tp[:, :])
                ag_dmas = []
                for m in range(4):
                    dma = nc.sync.dma_start(
                        out=agin[g][p][m * 128:(m + 1) * 128, :],
                        in_=agst[:, 2 * m * 128:(2 * m + 2) * 128],
                    )
                    if prev_cc[g][p] is not None:
                        tile.add_dep_helper(
                            dma.ins, prev_cc[g][p].ins, reason="agin WAR vs prev AG"
                        )
                    ag_dmas.append(dma)
                cc = nc.gpsimd.collective_compute(
                    "AllGather",
                    ALU.bypass,
                    replica_groups=[list(range(NCORES))],
                    ins=[agin[g][p][:, :].opt()],
                    outs=[xg[g][p][:, :].opt()],
                )
                for dma in ag_dmas:
                    tile.add_dep_helper(cc.ins, dma.ins, reason="AG RAW on agin")
                for dma in prev_din[g][p]:
                    tile.add_dep_helper(cc.ins, dma.ins, reason="xg WAR vs prev read")
                prev_cc[g][p] = cc
                # gathered state -> SBUF quarters
                base = xg[g][p][:, :]
                dins = []
                for j in range(4):
                    tq = xgq.tile([128, 8 * 256], FP16, tag=f"xg{g}q{j}")
                    if j == 0:
                        in_ap = bass.AP(
                            tensor=base.tensor,
                            offset=base.offset,
                            ap=[[256, 128], [1, 256]],
                        )
                        d0 = nc.sync.dma_start(out=tq[:, 0:256], in_=in_ap)
                        tile.add_dep_helper(d0.ins, cc.ins, reason="stream RAW")
                        dins.append(d0)
                        in_ap = bass.AP(
                            tensor=base.tensor,
                            offset=base.offset + 128 * 256,
                            ap=[[256, 128], [128 * 256, 7], [1, 256]],
                        )
                        dma = nc.sync.dma_start(out=tq[:, 256:], in_=in_ap)
                    else:
                        in_ap = bass.AP(
                            tensor=base.tensor,
                            offset=base.offset + j * 1024 * 256,
                            ap=[[256, 128], [128 * 256, 8], [1, 256]],
                        )
                        dma = nc.sync.dma_start(out=tq[:, :], in_=in_ap)
                    tile.add_dep_helper(dma.ins, cc.ins, reason="stream RAW on AG")
                    dins.append(dma)
                    cur_q[g][j] = tq
                prev_din[g][p] = dins

            def elementwise(g, u, v):
                """u,v PSUM [128,512] fp32 -> updated x tiles (in-place)."""
                x1, x2 = xs[g][0], xs[g][1]
                vs = tmp.tile([128, IK], FP32, tag="vs")
                S.copy(vs[:, :], v[:, :])
                h1 = tmp.tile([128, IK], FP32, tag="h1")
                h2 = tmp.tile([128, IK], FP32, tag="h2")
                V.tensor_add(out=h1[:, :], in0=u[:, :], in1=vs[:, :])
                V.tensor_sub(out=h2[:, :], in0=u[:, :], in1=vs[:, :])
                xf1 = tmp.tile([128, IK], FP32, tag="xf1")
                xf2 = tmp.tile([128, IK], FP32, tag="xf2")
                V.tensor_mul(out=xf1[:, :], in0=x1[:, :], in1=h1[:, :])
                V.tensor_mul(out=xf2[:, :], in0=x2[:, :], in1=h2[:, :])
                dot1 = tmp.tile([128, NI], FP32, tag="dot1")
                dot2 = tmp.tile([128, NI], FP32, tag="dot2")
                V.tensor_reduce(
                    out=dot1[:, :], in_=win3(xf1[:, :]),
                    axis=mybir.AxisListType.X, op=ALU.add,
                )
                V.tensor_reduce(
                    out=dot2[:, :], in_=win3(xf2[:, :]),
                    axis=mybir.AxisListType.X, op=ALU.add,
                )
                g1 = tmp.tile([128, NI], FP32, tag="g1")
                g2 = tmp.tile([128, NI], FP32, tag="g2")
                # g = 1 - gamma*dot
                V.tensor_scalar(
                    out=g1[:, :], in0=dot1[:, :], scalar1=-GAMMA,
                    scalar2=1.0, op0=ALU.mult, op1=ALU.add,
                )
                V.tensor_scalar(
                    out=g2[:, :], in0=dot2[:, :], scalar1=-GAMMA,
                    scalar2=1.0, op0=ALU.mult, op1=ALU.add,
                )
                xg1 = tmp.tile([128, IK], FP32, tag="xg1")
                xg2 = tmp.tile([128, IK], FP32, tag="xg2")
                V.tensor_mul(out=win3(xg1[:, :]), in0=win3(x1[:, :]), in1=bcast(g1))
                V.tensor_mul(out=win3(xg2[:, :]), in0=win3(x2[:, :]), in1=bcast(g2))
                pre1 = tmp.tile([128, IK], FP32, tag="pre1")
                pre2 = tmp.tile([128, IK], FP32, tag="pre2")
                V.scalar_tensor_tensor(
                    out=pre1[:, :], in0=h1[:, :], scalar=GAMMA, in1=xg1[:, :],
                    op0=ALU.mult, op1=ALU.add,
                )
                V.scalar_tensor_tensor(
                    out=pre2[:, :], in0=h2[:, :], scalar=GAMMA, in1=xg2[:, :],
                    op0=ALU.mult, op1=ALU.add,
                )
                _norm_apply(g, pre1, pre2)

            def _norm_apply(g, pre1, pre2):
                """x[g][st] = pre_st / ||pre_st|| (windowed over 16)."""
                x1, x2 = xs[g][0], xs[g][1]
                sq1 = tmp.tile([128, IK], FP32, tag="sq1")
                sq2 = tmp.tile([128, IK], FP32, tag="sq2")
                S.square(sq1[:, :], pre1[:, :])
                S.square(sq2[:, :], pre2[:, :])
                n21 = tmp.tile([128, NI], FP32, tag="n21")
                n22 = tmp.tile([128, NI], FP32, tag="n22")
                V.tensor_reduce(
                    out=n21[:, :], in_=win3(sq1[:, :]),
                    axis=mybir.AxisListType.X, op=ALU.add,
                )
                V.tensor_reduce(
                    out=n22[:, :], in_=win3(sq2[:, :]),
                    axis=mybir.AxisListType.X, op=ALU.add,
                )
                nrm1 = tmp.tile([128, NI], FP32, tag="nrm1")
                nrm2 = tmp.tile([128, NI], FP32, tag="nrm2")
                S.sqrt(nrm1[:, :], n21[:, :])
                S.sqrt(nrm2[:, :], n22[:, :])
                rv1 = tmp.tile([128, NI], FP32, tag="rv1")
                rv2 = tmp.tile([128, NI], FP32, tag="rv2")
                V.reciprocal(out=rv1[:, :], in_=nrm1[:, :])
                V.reciprocal(out=rv2[:, :], in_=nrm2[:, :])
                V.tensor_mul(out=win3(x1[:, :]), in0=win3(pre1[:, :]), in1=bcast(rv1))
                P.tensor_mul(out=win3(x2[:, :]), in0=win3(pre2[:, :]), in1=bcast(rv2))

            # ---------------- init: x0 = normalize(noise + c) ----------------
            for g in range(NG):
                for st in range(2):
                    cps = fps.tile([128, IK], FP32, tag=f"u{g}")
                    nc.tensor.matmul(
                        cps[:, :], ones_sb[:, :], bdr_sb[:, :],
                        start=True, stop=False, skip_group_check=True,
                    )
                    nc.tensor.matmul(
                        cps[:, :], emb_chunk(g, 2 + st), wbd_sb[:, :],
                        start=False, stop=True, skip_group_check=True,
                    )
                    nt = tmp.tile([128, IK], FP32, tag="noise")
                    nc.sync.dma_start(
                        out=nt[:, :],
                        in_=noise_d[g * 128:(g + 1) * 128, st * IK:(st + 1) * IK],
                    )
                    pre = tmp.tile([128, IK], FP32, tag=f"pre{st + 1}")
                    V.tensor_add(out=pre[:, :], in0=cps[:, :], in1=nt[:, :])
                    if st == 0:
                        pre1 = pre
                    else:
                        _norm_apply(g, pre1, pre)
                launch_gather(g, 0)

            # ---------------- main loop ----------------
            # Launch the previous group's transpose+AllGather tail at the TOP
            # of each block: its PE transposes stall only on that group's
            # elementwise (running right now on V/P), so the AG launches at
            # the earliest dependency-legal moment and has ~two full blocks
            # of cover before its consumer matmuls.
            pending_tail = None
            for t in range(T):
                for g in range(NG):
                    if pending_tail is not None:
                        launch_gather(*pending_tail)
                        pending_tail = None
                    u = fps.tile([128, IK], FP32, tag=f"u{g}")
                    v = fps.tile([128, IK], FP32, tag=f"v{g}")
                    # c-folds: u += b_d + emb_s (x) W ; v += emb_d (x) W
                    nc.tensor.matmul(
                        u[:, :], ones_sb[:, :], bdr_sb[:, :],
                        start=True, stop=False, skip_group_check=True,
                    )
                    nc.tensor.matmul(
                        u[:, :], emb_chunk(g, 0), wbd_sb[:, :],
                        start=False, stop=False, skip_group_check=True,
                    )
                    nc.tensor.matmul(
                        v[:, :], emb_chunk(g, 1), wbd_sb[:, :],
                        start=True, stop=False, skip_group_check=True,
                    )
                    for k in range(NCH):
                        tq = cur_q[g][k // 8]
                        c0 = (k % 8) * 256
                        s_chunk = tq[:, c0:c0 + 128]
                        d_chunk = tq[:, c0 + 128:c0 + 256]
                        last = k == NCH - 1
                        nc.tensor.matmul(
                            u[:, :], s_chunk, a_sb[:, k * IK:(k + 1) * IK],
                            start=False, stop=last, skip_group_check=True,
                        )
                        nc.tensor.matmul(
                            v[:, :], d_chunk, b_sb[:, k * IK:(k + 1) * IK],
                            start=False, stop=last, skip_group_check=True,
                        )
                    elementwise(g, u, v)
                    if t < T - 1:
                        pending_tail = (g, t + 1)

            # ---------------- output ----------------
            for g in range(NG):
                for st in range(2):
                    nc.sync.dma_start(
                        out=out_d[g * 128:(g + 1) * 128, st * IK:(st + 1) * IK],
                        in_=xs[g][st][:, :],
                    )

    nc.compile()
    return nc


def _get_nc():
    if "nc" not in _CACHE:
        nc = bacc.Bacc(
            "TRN2", target_bir_lowering=False, debug=False, num_devices=NCORES
        )
        _build(nc)
        nc.m = get_hw_module(nc.m)
        _CACHE["nc"] = nc
    return _CACHE["nc"]


def _marshal(embeddings1, embeddings2, W_d, b_d, J_in, J_out, Omega, noise1, noise2):
    """Host-side pure data movement + linear prep: build A/Bm slices etc."""
    f32 = np.float32
    AT = (J_in + J_out).transpose(1, 3, 0, 2).reshape(DN, DN).astype(f32) * 0.5
    BT2 = (J_in - J_out).transpose(1, 3, 0, 2).reshape(DN, DN).astype(f32) * 0.5
    for i in range(D):
        blk = 0.5 * Omega[i].T  # [l,k]
        AT[i * N:(i + 1) * N, i * N:(i + 1) * N] += blk
        BT2[i * N:(i + 1) * N, i * N:(i + 1) * N] += blk
    emb_s = 0.5 * (embeddings1 + embeddings2)
    emb_d = 0.5 * (embeddings1 - embeddings2)
    n1 = noise1.reshape(B, DN)
    n2 = noise2.reshape(B, DN)
    bd_flat = b_d.reshape(DN)

    in_maps = []
    for q in range(NCORES):
        ik0 = IK * q
        i0 = NI * q
        j0 = q // 4
        wbd = np.zeros((128, IK), f32)
        r0 = i0 - 128 * j0
        for il in range(NI):
            wbd[r0 + il, il * N:(il + 1) * N] = W_d[i0 + il]
        embs = np.zeros((128, NG * 4 * 128), f32)
        for g in range(NG):
            bsl = slice(128 * g, 128 * (g + 1))
            for kind, e in enumerate((emb_s, emb_d, embeddings1, embeddings2)):
                embs[:, (g * 4 + kind) * 128:(g * 4 + kind + 1) * 128] = (
                    e[bsl, 128 * j0:128 * (j0 + 1)].T
                )
        noise = np.concatenate(
            [n1[:, ik0:ik0 + IK], n2[:, ik0:ik0 + IK]], axis=1
        )
        in_maps.append(
            {
                "a_mat": np.ascontiguousarray(AT[:, ik0:ik0 + IK]).astype(np.float16),
                "b_mat": np.ascontiguousarray(BT2[:, ik0:ik0 + IK]).astype(np.float16),
                "wbd": wbd.astype(np.float16),
                "embs": embs.astype(np.float16),
                "ones_r": np.ones((1, 128), np.float16),
                "bd_r": bd_flat[ik0:ik0 + IK][None].astype(np.float16),
                "ident": np.eye(128, dtype=np.float16),
                "noise": np.ascontiguousarray(noise, f32),
            }
        )
    return in_maps


def _unmarshal(results):
    out = np.empty((2, B, D, N), np.float32)
    for q in range(NCORES):
        xt = results[q]["xt_out"]  # [256, 1024]
        i0 = NI * q
        out[0][:, i0:i0 + NI, :] = xt[:, :IK].reshape(B, NI, N)
        out[1][:, i0:i0 + NI, :] = xt[:, IK:].reshape(B, NI, N)
    return out


def run_on_device(in_maps):
    nc = _get_nc()
    return bass2jax.run_bass_via_pjrt(nc, in_maps, n_cores=NCORES)


def kernel(**inputs):
    in_maps = _marshal(**{k: np.asarray(v, np.float32) for k, v in inputs.items()})
    results = run_on_device(in_maps)
    return _unmarshal(results)


if __name__ == "__main__":
    rng = np.random.default_rng(0)
    ins = {
        "embeddings1": rng.standard_normal((B, D), dtype=np.float32),
        "embeddings2": rng.standard_normal((B, D), dtype=np.float32),
        "W_d": rng.standard_normal((D, N), dtype=np.float32) * 0.1,
        "b_d": np.zeros((D, N), np.float32),
        "J_in": (rng.standard_normal((D, D, N, N), dtype=np.float32) * 0.007),
        "J_out": (rng.standard_normal((D, D, N, N), dtype=np.float32) * 0.007),
        "Omega": rng.standard_normal((D, N, N), dtype=np.float32) * 0.1,
        "noise1": rng.standard_normal((B, D, N), dtype=np.float32) * 0.05,
        "noise2": rng.standard_normal((B, D, N), dtype=np.float32) * 0.05,
    }
    t0 = time.time()
    out = kernel(**ins)
    print("kernel() took", time.time() - t0, "s; out shape", out.shape)
